# revision 2
# baseline (speedup 1.0000x reference)
"""Hamming-distance kernel for Trainium2 (8 NeuronCores, SPMD) (final).

out[n, m] = mean_d(x[n, d] != y[m, d]),  x: (8192, 256), y: (8192, 256),
values small integers 0..7 stored as float32.

Formulation: categorical equality as a +-1 Hadamard-code GEMM.
dot[n,m] over K = 7*256 = 1792 features gives eq = (dot + 256)/8 and
out = 1 - eq/256.  Exact arithmetic; output ships as uint8 eq counts and
the host applies 1 - eq/256.

Key layout rule learned from traces: DMA throughput collapses (~45 GB/s)
when per-partition runs are 512 B; ≥1 KB contiguous runs reach ~90-107
GB/s.  So every DMA here moves ≥1 KB-contiguous-per-partition blocks:
  - x codes: host-precomputed fp8, dram [128, 7, 2, 2, 512] so each
    (kp, side) piece is one 1 KB run per partition.  14 pieces across
    three queues (Sync / Scalar / GpSimd), ordered by chunk-0 need time.
  - y raw: dram [128, 16, 2, 512] (chunk-blocked, 1 KB runs); per-chunk
    slices early, 4-chunk bulk pieces later.
  - output: evictions write uint8 eq into a [128, 2048] staging tile (4
    banks of a half-pass); ONE 2 KB-run DMA per half-pass.
Other structure:
  - Plane order [4,1,2,5,3,6,7]; plane 0 = Sign(3.5 - v) straight from
    raw fp8.  Each encode op covers both D-halves via a [128, 2, 512]
    AP (pairs adjacent in k).  All products on DVE.
  - Chunk 0: kp-outer over all 8 PSUM banks (relaxes the per-kp supply
    deadline to 1.73 us); chunks 1+: half-pass.  Chunk-0 evictions are
    interleaved ACT/DVE two-per-engine so the 8-bank turnaround into
    chunk 1 fits; later chunks evict on ACT (last chunk splits).
  - 7 warmup matmuls on a GpSimd-memset dummy start the HAM ramp ~7 us.
"""

import numpy as np
import ml_dtypes

import concourse.bacc as bacc
import concourse.mybir as mybir
import concourse.tile as tile
from concourse.bass_utils import run_bass_kernel_spmd

N, M, D, C = 8192, 8192, 256, 8
N_CORES = 8
N_SH = N // N_CORES

P = 128
D_HALVES = D // P  # 2
N_CODES = 7
KSUB = N_CODES * D_HALVES  # 14
K_PAIRS = KSUB // 2  # 7
M_CHUNK = 512
M_CHUNKS = M // M_CHUNK  # 16
N_TILES = N_SH // P  # 8
HALF = N_SH // 2  # 512

MASKS = (4, 1, 2, 5, 3, 6, 7)

FP8 = mybir.dt.float8e4
F32 = mybir.dt.float32
U8 = mybir.dt.uint8
I32 = mybir.dt.int32
ALU = mybir.AluOpType
ACTF = mybir.ActivationFunctionType
DR = mybir.MatmulPerfMode.DoubleRow

N_WARM = 7
YE_RING = 3


def _enc_chain(nc, tmp_pool, yt, raw, biases):
    """Encode one y chunk; each op covers BOTH D-halves of a plane pair
    ([128, 2, 512] APs).  Plane pair j = yt[:, 2j:2j+2, :]."""
    b05, b15, b35 = biases
    pair = lambda j: yt[:, 2 * j : 2 * j + 2, :]
    nc.scalar.activation(pair(0), raw, ACTF.Sign, bias=b35[:], scale=-1.0)
    yield
    vi = tmp_pool.tile([P, D_HALVES, M_CHUNK], I32, name="enc_vi")
    nc.vector.tensor_copy(vi[:], raw)
    yield
    t0 = tmp_pool.tile([P, D_HALVES, M_CHUNK], I32, name="enc_t0")
    nc.vector.tensor_scalar(
        out=t0[:], in0=vi[:], scalar1=1, scalar2=None, op0=ALU.bitwise_and
    )
    yield
    nc.scalar.activation(pair(1), t0[:], ACTF.Sign, bias=b05[:], scale=-1.0)
    yield
    u = tmp_pool.tile([P, D_HALVES, M_CHUNK], I32, name="enc_u")
    nc.vector.tensor_scalar(
        out=u[:], in0=vi[:], scalar1=2, scalar2=None, op0=ALU.bitwise_and
    )
    yield
    nc.scalar.activation(pair(2), u[:], ACTF.Sign, bias=b15[:], scale=-1.0)
    yield
    nc.vector.tensor_tensor(pair(3), pair(1), pair(0), ALU.mult)  # s5=s1*s4
    yield
    nc.vector.tensor_tensor(pair(4), pair(1), pair(2), ALU.mult)  # s3=s1*s2
    yield
    nc.vector.tensor_tensor(pair(5), pair(2), pair(0), ALU.mult)  # s6=s2*s4
    yield
    nc.vector.tensor_tensor(pair(6), pair(4), pair(0), ALU.mult)  # s7=s3*s4
    yield


def _drain(chains, steps):
    n = 0
    while chains and n < steps:
        ch = chains[0]
        try:
            next(ch)
            n += 1
        except StopIteration:
            chains.pop(0)
            continue
        chains.append(chains.pop(0))
    return chains


def _build_bass():
    nc = bacc.Bacc(
        "TRN2", target_bir_lowering=False, debug=False, num_devices=N_CORES
    )

    # x codes, piece-blocked: xe[p, j, s, h, t] = code_j(x[512s+t, 128h+p]).
    xe_d = nc.dram_tensor(
        "xe", [P, K_PAIRS, 2, D_HALVES, HALF], FP8, kind="ExternalInput"
    )
    # y raw, chunk-blocked: yf[p, c, h, t] = y[512c+t, 128h+p].
    yf_d = nc.dram_tensor(
        "yf", [P, M_CHUNKS, D_HALVES, M_CHUNK], FP8, kind="ExternalInput"
    )
    # Output: (mc, half) block = [128, 2048] uint8 eq counts
    # (4 n-tiles x 512 m-cols); host deblocks.
    out_d = nc.dram_tensor(
        "out", [M_CHUNKS, 2, P, 4 * M_CHUNK], U8, kind="ExternalOutput"
    )

    with tile.TileContext(nc) as tc:
        with (
            tc.tile_pool(name="warm", bufs=1) as warm_pool,
            tc.tile_pool(name="xe", bufs=1) as xe_pool,
            tc.tile_pool(name="yraw", bufs=1) as yraw_pool,
            tc.tile_pool(name="ye", bufs=YE_RING) as ye_pool,
            tc.tile_pool(name="tmp", bufs=6) as tmp_pool,
            tc.tile_pool(name="out", bufs=4) as out_pool,
            tc.tile_pool(name="psum", bufs=8, space="PSUM") as psum_pool,
        ):
            # ---- constants ----
            biases = []
            for val in (0.5, 1.5, 3.5):
                b = tmp_pool.tile([P, 1], F32, name=f"bias_{val}", bufs=1)
                nc.vector.memset(b[:], val)
                biases.append(b)
            wsig = warm_pool.tile([P, 1], F32)
            nc.scalar.activation(
                wsig[:], biases[1][:], ACTF.Sign, bias=biases[0][:], scale=-1.0
            )
            dw = warm_pool.tile([P, D_HALVES, M_CHUNK], FP8)
            nc.gpsimd.memset(dw[:], 1.0)

            wpsums = [
                psum_pool.tile([P, M_CHUNK], F32, name="psum") for _ in range(8)
            ]
            for i in range(N_WARM):
                nc.tensor.matmul(
                    wpsums[i % 8][:], dw[:, :, :P], dw[:], start=True,
                    stop=True, perf_mode=DR,
                )

            # ---- input DMAs (all pieces are 1KB+ contiguous runs) ----
            xe = xe_pool.tile([P, K_PAIRS, 2, D_HALVES, HALF], FP8)
            yraw = yraw_pool.tile([P, M_CHUNKS, D_HALVES, M_CHUNK], FP8)

            def xe_piece(eng, j, side):
                eng.dma_start(xe[:, j, side], xe_d[:, j, side])

            def y_chunks(eng, lo, hi):
                eng.dma_start(yraw[:, lo:hi], yf_d[:, lo:hi])

            # Scalar queue: the kp0 L piece (first-matmul gate), then y
            # chunk 1 (early so chunk-1's first sign can never head-of-line
            # block ACT); kp6 R woven between the chunk-0 signs (below).
            xe_piece(nc.scalar, 0, 0)
            y_chunks(nc.scalar, 1, 2)
            # Sync queue: y c0, then L/R pieces in need order, then bulk
            # y (chunks 4-15 in two pieces).
            y_chunks(nc.sync, 0, 1)
            xe_piece(nc.sync, 1, 0)
            xe_piece(nc.sync, 2, 0)
            xe_piece(nc.sync, 2, 1)
            xe_piece(nc.sync, 3, 0)
            xe_piece(nc.sync, 4, 0)
            xe_piece(nc.sync, 5, 0)
            xe_piece(nc.sync, 6, 0)
            y_chunks(nc.sync, 4, 10)
            y_chunks(nc.sync, 10, 16)
            # GpSimd queue (slow): R pieces + y chunks 2-3.
            xe_piece(nc.gpsimd, 0, 1)
            xe_piece(nc.gpsimd, 1, 1)
            y_chunks(nc.gpsimd, 2, 3)
            xe_piece(nc.gpsimd, 3, 1)
            xe_piece(nc.gpsimd, 4, 1)
            xe_piece(nc.gpsimd, 5, 1)
            y_chunks(nc.gpsimd, 3, 4)

            # ---- encoded-y chunk ring ----
            ye_tiles = [None] * M_CHUNKS

            def start_y_chunk(c):
                yt = ye_pool.tile([P, KSUB, M_CHUNK], FP8, name="ye")
                ye_tiles[c] = yt
                raw = yraw[:, c]
                return [_enc_chain(nc, tmp_pool, yt, raw, biases)]

            c0 = start_y_chunk(0)
            _drain(c0, 5)  # s4 pair (first-matmul gate), cast, AND1, s1, AND2
            xe_piece(nc.scalar, 6, 1)
            _drain(c0, 10_000)
            _drain(start_y_chunk(1), 10_000)

            # ---- main loop ----
            def mk_lhs(kp, n):
                return xe[:, kp, n // 4, :, (n % 4) * P : (n % 4 + 1) * P]

            pending = []
            for mc in range(M_CHUNKS):
                if mc + 2 < M_CHUNKS:
                    pending += start_y_chunk(mc + 2)
                yt = ye_tiles[mc]
                last = mc == M_CHUNKS - 1

                def evict(st, nn, psrc, use_dve):
                    # eq = dot/8 + 32  (exact integer in fp32)
                    dst = st[:, nn * M_CHUNK : (nn + 1) * M_CHUNK]
                    if use_dve:
                        nc.vector.tensor_scalar(
                            out=dst, in0=psrc[:], scalar1=0.125, scalar2=32.0,
                            op0=ALU.mult, op1=ALU.add,
                        )
                    else:
                        nc.scalar.activation(
                            dst, psrc[:], ACTF.Copy, bias=32.0, scale=0.125
                        )

                if mc == 0:
                    # Full pass: kp-outer over all 8 n-tiles / PSUM banks.
                    # No pending drains inside: chunk-2 encode must queue
                    # behind these evictions on ACT/DVE.
                    psums = [
                        psum_pool.tile([P, M_CHUNK], F32, name="psum")
                        for _ in range(8)
                    ]
                    for kp in range(K_PAIRS):
                        for n in range(8):
                            nc.tensor.matmul(
                                psums[n][:], mk_lhs(kp, n),
                                yt[:, 2 * kp : 2 * kp + 2, :],
                                start=(kp == 0), stop=(kp == K_PAIRS - 1),
                                perf_mode=DR,
                            )
                    sts = [
                        out_pool.tile([P, 4 * M_CHUNK], U8, name="st")
                        for _ in range(2)
                    ]
                    # Banks 0-3 first (chunk-1 half 0), two per engine.
                    for n, dve in ((0, False), (1, True), (2, False),
                                   (3, True), (4, False), (5, True),
                                   (6, False), (7, True)):
                        evict(sts[n // 4], n % 4, psums[n], dve)
                        if n == 3:
                            nc.sync.dma_start(out_d[0, 0], sts[0][:])
                    nc.sync.dma_start(out_d[0, 1], sts[1][:])
                elif last:
                    # Final chunk: bank-serial (kp-inner) so each bank's
                    # eviction + output piece pipelines out immediately;
                    # the tail after the last matmul is one eviction plus
                    # one 64 KB DMA instead of a 4-bank turnaround.
                    engs = (nc.sync, nc.gpsimd, nc.scalar)
                    for n in range(8):
                        ps = psum_pool.tile([P, M_CHUNK], F32, name="psum")
                        for kp in range(K_PAIRS):
                            nc.tensor.matmul(
                                ps[:], mk_lhs(kp, n),
                                yt[:, 2 * kp : 2 * kp + 2, :],
                                start=(kp == 0), stop=(kp == K_PAIRS - 1),
                                perf_mode=DR,
                            )
                        st = out_pool.tile([P, M_CHUNK], U8, name="stl")
                        # alternate ACT/DVE per bank for the fastest drain
                        if n % 2 == 0:
                            nc.scalar.activation(
                                st[:], ps[:], ACTF.Copy, bias=32.0, scale=0.125
                            )
                        else:
                            nc.vector.tensor_scalar(
                                out=st[:], in0=ps[:], scalar1=0.125,
                                scalar2=32.0, op0=ALU.mult, op1=ALU.add,
                            )
                        half, nn = n // 4, n % 4
                        engs[n % 3].dma_start(
                            out_d[mc, half, :,
                                  nn * M_CHUNK : (nn + 1) * M_CHUNK],
                            st[:],
                        )
                else:
                    for half in range(2):
                        psums = [
                            psum_pool.tile([P, M_CHUNK], F32, name="psum")
                            for _ in range(4)
                        ]
                        for kp in range(K_PAIRS):
                            for nn in range(4):
                                nc.tensor.matmul(
                                    psums[nn][:], mk_lhs(kp, 4 * half + nn),
                                    yt[:, 2 * kp : 2 * kp + 2, :],
                                    start=(kp == 0), stop=(kp == K_PAIRS - 1),
                                    perf_mode=DR,
                                )
                            pending = _drain(pending, 1)
                        st = out_pool.tile([P, 4 * M_CHUNK], U8, name="st")
                        for nn in range(4):
                            evict(st, nn, psums[nn], nn == 1)
                        nc.sync.dma_start(out_d[mc, half], st[:])
                _drain(pending, 6 if mc == 0 else 4)
    nc.compile()
    return nc


_NC_CACHE = {}


def _get_nc():
    if "nc" not in _NC_CACHE:
        _NC_CACHE["nc"] = _build_bass()
    return _NC_CACHE["nc"]


def _pack_y(t: np.ndarray) -> np.ndarray:
    """(8192, 256) -> fp8 [128, 16, 2, 512]: out[p,c,h,u] = t[512c+u, 128h+p]."""
    tt = np.ascontiguousarray(t.T)  # (256, 8192) [d, m]
    r = tt.reshape(D_HALVES, P, M_CHUNKS, M_CHUNK)  # [h, p, c, u]
    return np.ascontiguousarray(r.transpose(1, 2, 0, 3)).astype(
        ml_dtypes.float8_e4m3fn
    )


_SGN = None


def _sgn_table():
    global _SGN
    if _SGN is None:
        s = np.empty((N_CODES, C), np.float32)
        for j, m in enumerate(MASKS):
            for v in range(C):
                s[j, v] = -1.0 if bin(v & m).count("1") % 2 else 1.0
        _SGN = s
    return _SGN


def _pack_x_codes(xs: np.ndarray) -> np.ndarray:
    """(1024, 256) -> fp8 [128, 7, 2, 2, 512]:
    out[p, j, s, h, u] = code_j(xs[512s+u, 128h+p])."""
    v = xs.astype(np.int32)
    codes = _sgn_table()[:, v]  # (7, 1024, 256) [j, i, d]
    c = codes.reshape(N_CODES, 2, HALF, D_HALVES, P)  # [j, s, u, h, p]
    xe = c.transpose(4, 0, 1, 3, 2)  # [p, j, s, h, u]
    return np.ascontiguousarray(xe).astype(ml_dtypes.float8_e4m3fn)


def _make_in_maps(x: np.ndarray, y: np.ndarray):
    yr = _pack_y(y)
    in_maps = []
    for i in range(N_CORES):
        xe = _pack_x_codes(x[i * N_SH : (i + 1) * N_SH])
        in_maps.append({"xe": xe, "yf": yr})
    return in_maps


def _deblock(blocked: np.ndarray) -> np.ndarray:
    # (16, 2, 128, 2048) u8 -> (1024, 8192) f32 distances.
    b = blocked.reshape(M_CHUNKS, 2, P, 4, M_CHUNK)  # [mc, half, p, nn, u]
    eq = (
        b.transpose(1, 3, 2, 0, 4)  # [half, nn, p, mc, u]
        .reshape(N_SH, M)
        .astype(np.float32)
    )
    return 1.0 - eq * np.float32(1.0 / 256.0)


def kernel(x: np.ndarray, y: np.ndarray, _trace: bool = False):
    x = np.asarray(x, dtype=np.float32)
    y = np.asarray(y, dtype=np.float32)
    assert x.shape == (N, D) and y.shape == (M, D)

    nc = _get_nc()
    in_maps = _make_in_maps(x, y)
    res = run_bass_kernel_spmd(
        nc, in_maps, core_ids=list(range(N_CORES)), trace=_trace
    )
    out = np.concatenate(
        [_deblock(np.asarray(r["out"])) for r in res.results], axis=0
    )
    if _trace:
        return out, res
    return out


# revision 3
# speedup vs baseline: 1.1992x; 1.1992x over previous
"""Hamming-distance kernel for Trainium2 (8 NeuronCores, SPMD) (final).

out[n, m] = mean_d(x[n, d] != y[m, d]),  x: (8192, 256), y: (8192, 256),
values small integers 0..7 stored as float32.

Formulation: categorical equality as a +-1 Hadamard-code GEMM.
dot[n,m] over K = 7*256 = 1792 features gives eq = (dot + 256)/8 and
out = 1 - eq/256.  Exact arithmetic; output ships as uint8 eq counts and
the host applies 1 - eq/256.

Key layout rule learned from traces: DMA throughput collapses (~45 GB/s)
when per-partition runs are 512 B; ≥1 KB contiguous runs reach ~90-107
GB/s.  So every DMA here moves ≥1 KB-contiguous-per-partition blocks:
  - x codes: host-precomputed fp8, dram [128, 7, 2, 2, 512] so each
    (kp, side) piece is one 1 KB run per partition.  14 pieces across
    three queues (Sync / Scalar / GpSimd), ordered by chunk-0 need time.
  - y raw: dram [128, 16, 2, 512] (chunk-blocked, 1 KB runs); per-chunk
    slices early, 4-chunk bulk pieces later.
  - output: evictions write uint8 eq into a [128, 2048] staging tile (4
    banks of a half-pass); ONE 2 KB-run DMA per half-pass.
Other structure:
  - Plane order [4,1,2,5,3,6,7]; plane 0 = Sign(3.5 - v) straight from
    raw fp8.  Each encode op covers both D-halves via a [128, 2, 512]
    AP (pairs adjacent in k).  All products on DVE.
  - Chunk 0: kp-outer over all 8 PSUM banks (relaxes the per-kp supply
    deadline to 1.73 us); chunks 1+: half-pass.  Chunk-0 evictions are
    interleaved ACT/DVE two-per-engine so the 8-bank turnaround into
    chunk 1 fits; later chunks evict on ACT (last chunk splits).
  - 7 warmup matmuls on a GpSimd-memset dummy start the HAM ramp ~7 us.
"""

import numpy as np
import ml_dtypes

import concourse.bacc as bacc
import concourse.mybir as mybir
import concourse.tile as tile
from concourse.bass_utils import run_bass_kernel_spmd

N, M, D, C = 8192, 8192, 256, 8
N_CORES = 8
N_SH = N // N_CORES

P = 128
D_HALVES = D // P  # 2
N_CODES = 7
KSUB = N_CODES * D_HALVES  # 14
K_PAIRS = KSUB // 2  # 7
M_CHUNK = 512
M_CHUNKS = M // M_CHUNK  # 16
N_TILES = N_SH // P  # 8
HALF = N_SH // 2  # 512

MASKS = (4, 1, 2, 5, 3, 6, 7)

FP8 = mybir.dt.float8e4
F32 = mybir.dt.float32
U8 = mybir.dt.uint8
I32 = mybir.dt.int32
ALU = mybir.AluOpType
ACTF = mybir.ActivationFunctionType
DR = mybir.MatmulPerfMode.DoubleRow

N_WARM = 7
YE_RING = 3


def _enc_chain(nc, tmp_pool, yt, raw, biases):
    """Encode one y chunk; each op covers BOTH D-halves of a plane pair
    ([128, 2, 512] APs).  Plane pair j = yt[:, 2j:2j+2, :]."""
    b05, b15, b35 = biases
    pair = lambda j: yt[:, 2 * j : 2 * j + 2, :]
    nc.scalar.activation(pair(0), raw, ACTF.Sign, bias=b35[:], scale=-1.0)
    yield
    vi = tmp_pool.tile([P, D_HALVES, M_CHUNK], I32, name="enc_vi")
    nc.vector.tensor_copy(vi[:], raw)
    yield
    t0 = tmp_pool.tile([P, D_HALVES, M_CHUNK], I32, name="enc_t0")
    nc.vector.tensor_scalar(
        out=t0[:], in0=vi[:], scalar1=1, scalar2=None, op0=ALU.bitwise_and
    )
    yield
    nc.scalar.activation(pair(1), t0[:], ACTF.Sign, bias=b05[:], scale=-1.0)
    yield
    u = tmp_pool.tile([P, D_HALVES, M_CHUNK], I32, name="enc_u")
    nc.vector.tensor_scalar(
        out=u[:], in0=vi[:], scalar1=2, scalar2=None, op0=ALU.bitwise_and
    )
    yield
    nc.scalar.activation(pair(2), u[:], ACTF.Sign, bias=b15[:], scale=-1.0)
    yield
    nc.vector.tensor_tensor(pair(3), pair(1), pair(0), ALU.mult)  # s5=s1*s4
    yield
    nc.vector.tensor_tensor(pair(4), pair(1), pair(2), ALU.mult)  # s3=s1*s2
    yield
    nc.vector.tensor_tensor(pair(5), pair(2), pair(0), ALU.mult)  # s6=s2*s4
    yield
    nc.vector.tensor_tensor(pair(6), pair(4), pair(0), ALU.mult)  # s7=s3*s4
    yield


def _drain(chains, steps):
    n = 0
    while chains and n < steps:
        ch = chains[0]
        try:
            next(ch)
            n += 1
        except StopIteration:
            chains.pop(0)
            continue
        chains.append(chains.pop(0))
    return chains


def _build_bass():
    nc = bacc.Bacc(
        "TRN2", target_bir_lowering=False, debug=False, num_devices=N_CORES
    )

    # x codes, piece-blocked: xe[p, j, s, h, t] = code_j(x[512s+t, 128h+p]).
    xe_d = nc.dram_tensor(
        "xe", [P, K_PAIRS, 2, D_HALVES, HALF], FP8, kind="ExternalInput"
    )
    # y raw, chunk-blocked: yf[p, c, h, t] = y[512c+t, 128h+p].
    yf_d = nc.dram_tensor(
        "yf", [P, M_CHUNKS, D_HALVES, M_CHUNK], FP8, kind="ExternalInput"
    )
    # Output: (mc, half) block = [128, 2048] uint8 eq counts
    # (4 n-tiles x 512 m-cols); host deblocks.
    out_d = nc.dram_tensor(
        "out", [M_CHUNKS, 2, P, 4 * M_CHUNK], U8, kind="ExternalOutput"
    )

    with tile.TileContext(nc) as tc:
        with (
            tc.tile_pool(name="warm", bufs=1) as warm_pool,
            tc.tile_pool(name="xe", bufs=1) as xe_pool,
            tc.tile_pool(name="yraw", bufs=1) as yraw_pool,
            tc.tile_pool(name="ye", bufs=YE_RING) as ye_pool,
            tc.tile_pool(name="tmp", bufs=6) as tmp_pool,
            tc.tile_pool(name="out", bufs=4) as out_pool,
            tc.tile_pool(name="psum", bufs=8, space="PSUM") as psum_pool,
        ):
            # ---- constants ----
            biases = []
            for val in (0.5, 1.5, 3.5):
                b = tmp_pool.tile([P, 1], F32, name=f"bias_{val}", bufs=1)
                nc.vector.memset(b[:], val)
                biases.append(b)
            wsig = warm_pool.tile([P, 1], F32)
            nc.scalar.activation(
                wsig[:], biases[1][:], ACTF.Sign, bias=biases[0][:], scale=-1.0
            )
            dw = warm_pool.tile([P, D_HALVES, M_CHUNK], FP8)
            nc.gpsimd.memset(dw[:], 1.0)

            wpsums = [
                psum_pool.tile([P, M_CHUNK], F32, name="psum") for _ in range(8)
            ]
            for i in range(N_WARM):
                nc.tensor.matmul(
                    wpsums[i % 8][:], dw[:, :, :P], dw[:], start=True,
                    stop=True, perf_mode=DR,
                )

            # ---- input DMAs (all pieces are 1KB+ contiguous runs) ----
            xe = xe_pool.tile([P, K_PAIRS, 2, D_HALVES, HALF], FP8)
            yraw = yraw_pool.tile([P, M_CHUNKS, D_HALVES, M_CHUNK], FP8)

            def xe_piece(eng, j, side):
                eng.dma_start(xe[:, j, side], xe_d[:, j, side])

            def y_chunks(eng, lo, hi):
                eng.dma_start(yraw[:, lo:hi], yf_d[:, lo:hi])

            # Scalar queue: the kp0 L piece (first-matmul gate), then y
            # chunk 1 (early so chunk-1's first sign can never head-of-line
            # block ACT); kp6 R woven between the chunk-0 signs (below).
            xe_piece(nc.scalar, 0, 0)
            y_chunks(nc.scalar, 1, 2)
            # Sync queue: y c0, then L/R pieces in need order, then bulk
            # y (chunks 4-15 in two pieces).
            y_chunks(nc.sync, 0, 1)
            xe_piece(nc.sync, 1, 0)
            xe_piece(nc.sync, 2, 0)
            xe_piece(nc.sync, 2, 1)
            xe_piece(nc.sync, 3, 0)
            xe_piece(nc.sync, 4, 0)
            xe_piece(nc.sync, 5, 0)
            xe_piece(nc.sync, 6, 0)
            y_chunks(nc.sync, 4, 10)
            y_chunks(nc.sync, 10, 16)
            # GpSimd queue (slow): R pieces + y chunks 2-3.
            xe_piece(nc.gpsimd, 0, 1)
            xe_piece(nc.gpsimd, 1, 1)
            y_chunks(nc.gpsimd, 2, 3)
            xe_piece(nc.gpsimd, 3, 1)
            xe_piece(nc.gpsimd, 4, 1)
            xe_piece(nc.gpsimd, 5, 1)
            y_chunks(nc.gpsimd, 3, 4)

            # ---- encoded-y chunk ring ----
            ye_tiles = [None] * M_CHUNKS

            def start_y_chunk(c):
                yt = ye_pool.tile([P, KSUB, M_CHUNK], FP8, name="ye")
                ye_tiles[c] = yt
                raw = yraw[:, c]
                return [_enc_chain(nc, tmp_pool, yt, raw, biases)]

            c0 = start_y_chunk(0)
            _drain(c0, 5)  # s4 pair (first-matmul gate), cast, AND1, s1, AND2
            xe_piece(nc.scalar, 6, 1)
            _drain(c0, 10_000)
            _drain(start_y_chunk(1), 10_000)

            # ---- main loop ----
            def mk_lhs(kp, n):
                return xe[:, kp, n // 4, :, (n % 4) * P : (n % 4 + 1) * P]

            pending = []
            for mc in range(M_CHUNKS):
                if mc + 2 < M_CHUNKS:
                    pending += start_y_chunk(mc + 2)
                yt = ye_tiles[mc]
                last = mc == M_CHUNKS - 1

                def evict(st, nn, psrc, use_dve):
                    # eq = dot/8 + 32  (exact integer in fp32)
                    dst = st[:, nn * M_CHUNK : (nn + 1) * M_CHUNK]
                    if use_dve:
                        nc.vector.tensor_scalar(
                            out=dst, in0=psrc[:], scalar1=0.125, scalar2=32.0,
                            op0=ALU.mult, op1=ALU.add,
                        )
                    else:
                        nc.scalar.activation(
                            dst, psrc[:], ACTF.Copy, bias=32.0, scale=0.125
                        )

                if mc == 0:
                    # Full pass: kp-outer over all 8 n-tiles / PSUM banks.
                    # No pending drains inside: chunk-2 encode must queue
                    # behind these evictions on ACT/DVE.
                    psums = [
                        psum_pool.tile([P, M_CHUNK], F32, name="psum")
                        for _ in range(8)
                    ]
                    for kp in range(K_PAIRS):
                        for n in range(8):
                            nc.tensor.matmul(
                                psums[n][:], mk_lhs(kp, n),
                                yt[:, 2 * kp : 2 * kp + 2, :],
                                start=(kp == 0), stop=(kp == K_PAIRS - 1),
                                perf_mode=DR,
                            )
                    sts = [
                        out_pool.tile([P, 4 * M_CHUNK], U8, name="st")
                        for _ in range(2)
                    ]
                    # Banks 0-3 first (chunk-1 half 0), two per engine.
                    for n, dve in ((0, False), (1, True), (2, False),
                                   (3, True), (4, False), (5, True),
                                   (6, False), (7, True)):
                        evict(sts[n // 4], n % 4, psums[n], dve)
                        if n == 3:
                            nc.sync.dma_start(out_d[0, 0], sts[0][:])
                    nc.sync.dma_start(out_d[0, 1], sts[1][:])
                elif last:
                    # Final chunk: bank-serial (kp-inner) so each bank's
                    # eviction + output piece pipelines out immediately;
                    # the tail after the last matmul is one eviction plus
                    # one 64 KB DMA instead of a 4-bank turnaround.
                    engs = (nc.sync, nc.gpsimd, nc.scalar)
                    for n in range(8):
                        ps = psum_pool.tile([P, M_CHUNK], F32, name="psum")
                        for kp in range(K_PAIRS):
                            nc.tensor.matmul(
                                ps[:], mk_lhs(kp, n),
                                yt[:, 2 * kp : 2 * kp + 2, :],
                                start=(kp == 0), stop=(kp == K_PAIRS - 1),
                                perf_mode=DR,
                            )
                        st = out_pool.tile([P, M_CHUNK], U8, name="stl")
                        # alternate ACT/DVE per bank for the fastest drain
                        if n % 2 == 0:
                            nc.scalar.activation(
                                st[:], ps[:], ACTF.Copy, bias=32.0, scale=0.125
                            )
                        else:
                            nc.vector.tensor_scalar(
                                out=st[:], in0=ps[:], scalar1=0.125,
                                scalar2=32.0, op0=ALU.mult, op1=ALU.add,
                            )
                        half, nn = n // 4, n % 4
                        engs[n % 3].dma_start(
                            out_d[mc, half, :,
                                  nn * M_CHUNK : (nn + 1) * M_CHUNK],
                            st[:],
                        )
                else:
                    for half in range(2):
                        psums = [
                            psum_pool.tile([P, M_CHUNK], F32, name="psum")
                            for _ in range(4)
                        ]
                        for kp in range(K_PAIRS):
                            for nn in range(4):
                                nc.tensor.matmul(
                                    psums[nn][:], mk_lhs(kp, 4 * half + nn),
                                    yt[:, 2 * kp : 2 * kp + 2, :],
                                    start=(kp == 0), stop=(kp == K_PAIRS - 1),
                                    perf_mode=DR,
                                )
                            pending = _drain(pending, 2)
                        st = out_pool.tile([P, 4 * M_CHUNK], U8, name="st")
                        for nn in range(4):
                            evict(st, nn, psums[nn], nn == 1)
                        nc.sync.dma_start(out_d[mc, half], st[:])
                _drain(pending, 6 if mc == 0 else 4)
    nc.compile()
    return nc


_NC_CACHE = {}


def _get_nc():
    if "nc" not in _NC_CACHE:
        _NC_CACHE["nc"] = _build_bass()
    return _NC_CACHE["nc"]


def _pack_y(t: np.ndarray) -> np.ndarray:
    """(8192, 256) -> fp8 [128, 16, 2, 512]: out[p,c,h,u] = t[512c+u, 128h+p]."""
    tt = np.ascontiguousarray(t.T)  # (256, 8192) [d, m]
    r = tt.reshape(D_HALVES, P, M_CHUNKS, M_CHUNK)  # [h, p, c, u]
    return np.ascontiguousarray(r.transpose(1, 2, 0, 3)).astype(
        ml_dtypes.float8_e4m3fn
    )


_SGN = None


def _sgn_table():
    global _SGN
    if _SGN is None:
        s = np.empty((N_CODES, C), np.float32)
        for j, m in enumerate(MASKS):
            for v in range(C):
                s[j, v] = -1.0 if bin(v & m).count("1") % 2 else 1.0
        _SGN = s
    return _SGN


def _pack_x_codes(xs: np.ndarray) -> np.ndarray:
    """(1024, 256) -> fp8 [128, 7, 2, 2, 512]:
    out[p, j, s, h, u] = code_j(xs[512s+u, 128h+p])."""
    v = xs.astype(np.int32)
    codes = _sgn_table()[:, v]  # (7, 1024, 256) [j, i, d]
    c = codes.reshape(N_CODES, 2, HALF, D_HALVES, P)  # [j, s, u, h, p]
    xe = c.transpose(4, 0, 1, 3, 2)  # [p, j, s, h, u]
    return np.ascontiguousarray(xe).astype(ml_dtypes.float8_e4m3fn)


def _make_in_maps(x: np.ndarray, y: np.ndarray):
    yr = _pack_y(y)
    in_maps = []
    for i in range(N_CORES):
        xe = _pack_x_codes(x[i * N_SH : (i + 1) * N_SH])
        in_maps.append({"xe": xe, "yf": yr})
    return in_maps


def _deblock(blocked: np.ndarray) -> np.ndarray:
    # (16, 2, 128, 2048) u8 -> (1024, 8192) f32 distances.
    b = blocked.reshape(M_CHUNKS, 2, P, 4, M_CHUNK)  # [mc, half, p, nn, u]
    eq = (
        b.transpose(1, 3, 2, 0, 4)  # [half, nn, p, mc, u]
        .reshape(N_SH, M)
        .astype(np.float32)
    )
    return 1.0 - eq * np.float32(1.0 / 256.0)


def kernel(x: np.ndarray, y: np.ndarray, _trace: bool = False):
    x = np.asarray(x, dtype=np.float32)
    y = np.asarray(y, dtype=np.float32)
    assert x.shape == (N, D) and y.shape == (M, D)

    nc = _get_nc()
    in_maps = _make_in_maps(x, y)
    res = run_bass_kernel_spmd(
        nc, in_maps, core_ids=list(range(N_CORES)), trace=_trace
    )
    out = np.concatenate(
        [_deblock(np.asarray(r["out"])) for r in res.results], axis=0
    )
    if _trace:
        return out, res
    return out
